# revision 1
# baseline (speedup 1.0000x reference)
"""Channel-attention (CAM) Trainium2 kernel.

Problem: out[b] = softmax(b_f[b] @ c_f[b].T, axis=-1) @ a_f[b] + a_f[b]
with a,b,c: [16, 1024, 32, 32] fp32, flattened to [16, 1024, 1024].

Sharding: pure data parallel over batch — 16 samples / 8 cores = 2 per core.

Per-core pipeline (per sample), fp16 compute:
  - b,c loaded via gpsimd cast-DMA (DRAM fp32 -> SBUF fp16), then
    PE-transposed (fp16) into the merged [HW, C] operand tensor bcT
  - m1: scores = bT.T @ cT, fp16 x1 (or x3 hi/lo split for high accuracy),
    fp32 PSUM accumulation
  - softmax: DVE row-max along free dim, ACT Exp with bias=-max and
    accum_out row-sum; the 1/sum division is deferred to the output
  - E (fp16) PE-transposed into ET (stationary operand of m2)
  - m2: out = ET.T @ a16, fp32 PSUM accumulation
  - finalize: one DVE scalar_tensor_tensor: out = psum * (1/sum) + a_fp32

Note: PE never executes fp32 ops — fp32 transpose-mode matmuls were
observed to hang the PE intermittently when interleaved with 16-bit
FWL-eligible matmul streams.
"""
import os
import sys
import types

import numpy as np


def _install_axon_hooks():
    """Provide antenv.axon_hooks (missing in this image) so trace=True works."""
    if 'antenv.axon_hooks' in sys.modules:
        return
    m = types.ModuleType('antenv.axon_hooks')
    m._hook = None
    m.set_axon_ntff_profile_hook = lambda h: setattr(m, '_hook', h)
    m.get_axon_ntff_profile_hook = lambda: m._hook
    sys.modules['antenv.axon_hooks'] = m
    try:
        import antenv
        antenv.axon_hooks = m
    except ImportError:
        pass
    try:
        from trn_agent_boot.trn_boot import _ntff_profile_via_ctypes
        m.set_axon_ntff_profile_hook(
            _ntff_profile_via_ctypes('/opt/axon/libaxon_pjrt.so'))
    except Exception:
        pass


_install_axon_hooks()

import concourse.bass as bass  # noqa: E402
import concourse.mybir as mybir  # noqa: E402
import concourse.tile as tile  # noqa: E402
from concourse import bacc, bass_utils  # noqa: E402
from concourse.masks import make_identity  # noqa: E402
from concourse.tile_rust import add_dep_helper  # noqa: E402

# artifact upload needs a bucket; keep everything local in the sandbox
bass_utils.upload_artifacts = lambda tmpdir: f"local:{tmpdir}"

N_CORES = 8
B, C, H, W = 16, 1024, 32, 32
HW = H * W
S = B // N_CORES        # samples per core
P = 128
NT = C // P             # 8 row tiles
F32 = mybir.dt.float32
F16 = mybir.dt.float16
ALU = mybir.AluOpType
AX = mybir.AxisListType
ACTF = mybir.ActivationFunctionType

# 1 = single-pass fp16 m1 (fast); 3 = fp16 hi/lo x3 m1 (high accuracy)
M1_TERMS = int(os.environ.get("CAM_M1_TERMS", "1"))


def cam_kernel(ctx, tc, out_ap, a_ap, b_ap, c_ap, n_samples=S):
    nc = tc.nc

    const_pool = ctx.enter_context(tc.tile_pool(name="const", bufs=1))
    big = ctx.enter_context(tc.tile_pool(name="big", bufs=1))
    big2 = ctx.enter_context(tc.tile_pool(name="big2", bufs=2))
    stg_pool = ctx.enter_context(tc.tile_pool(name="stage", bufs=4))
    epool = ctx.enter_context(tc.tile_pool(name="epool", bufs=2))
    opool = ctx.enter_context(tc.tile_pool(name="opool", bufs=2))
    arpool = ctx.enter_context(tc.tile_pool(name="ar", bufs=3))
    sm = ctx.enter_context(tc.tile_pool(name="sm", bufs=16))
    dram = ctx.enter_context(tc.tile_pool(name="dram", bufs=2, space="DRAM"))
    psum_t = ctx.enter_context(tc.tile_pool(name="psum_t", bufs=1, space="PSUM"))
    psum_s = ctx.enter_context(tc.tile_pool(name="psum_s", bufs=4, space="PSUM"))
    psum_o = ctx.enter_context(tc.tile_pool(name="psum_o", bufs=3, space="PSUM"))

    ident = const_pool.tile([P, P], F16)
    make_identity(nc, ident[:])

    for s in range(n_samples):
        a16 = big2.tile([P, NT, HW], F16, tag="a16")
        ET = big.tile([P, NT, C], F16, tag="ET")

        # ---- b,c -> fp16 transposed operands via cast-load + PE transpose ----
        # bcT free-dim layout (units of C): x1: [b, c]; x3: [b_hi, b_lo,
        # c_hi, c_lo]. m1_ops = (lhs col base, rhs col base) pairs.
        if M1_TERMS == 1:
            NW = 2
            bcT = big2.tile([P, NT, NW * C], F16, tag="bcT")
            srcs = [(b_ap, 0, None), (c_ap, C, None)]
            m1_ops = [(0, C)]
        else:
            NW = 4
            bcT = big.tile([P, NT, NW * C], F16, tag="bcT3")
            srcs = [(b_ap, 0, 1), (c_ap, 2 * C, 3)]
            m1_ops = [(0, 2 * C), (0, 3 * C), (C, 2 * C)]
        # order: b row 0, then all of c (m1 i=0 needs full cT), then b rest
        order = [(srcs[0], 0)] + [(srcs[1], r) for r in range(NT)] + \
                [(srcs[0], r) for r in range(1, NT)]
        for (src_ap, base, lo_q), r in order:
                rsl = slice(r * P, (r + 1) * P)
                nat = stg_pool.tile([P, HW], F16, tag="nat")
                nc.gpsimd.dma_start(nat[:], src_ap[s, rsl, :])  # cast f32->f16
                if lo_q is not None:
                    st = stg_pool.tile([P, HW], F32, tag="stage")
                    nc.sync.dma_start(st[:], src_ap[s, rsl, :])
                    lon = stg_pool.tile([P, HW], F16, tag="lon")
                    nc.vector.tensor_tensor(lon[:], st[:], nat[:],
                                            ALU.subtract)
                    nats = ((nat, base), (lon, base + C))
                else:
                    nats = ((nat, base),)
                for nt_, nb in nats:
                    pt = psum_t.tile([P, 8 * P], F16, tag="pt")
                    for j in range(NT):
                        nc.tensor.transpose(
                            pt[:, j * P:(j + 1) * P],
                            nt_[:, j * P:(j + 1) * P], ident[:])
                    nc.vector.tensor_copy(
                        bcT[:, :, nb + r * P:nb + (r + 1) * P],
                        pt[:].rearrange("p (t c) -> p t c", t=8))

        # ---- a -> fp16 via cast-DMA (m2 moving operand) ----
        for r in range(NT):
            nc.gpsimd.dma_start(a16[:, r, :], a_ap[s, r * P:(r + 1) * P, :])

        # ---- per output row-tile: m1, softmax, E^T, m2, finalize ----
        for i in range(NT):
            isl = slice(i * P, (i + 1) * P)
            ps0 = psum_s.tile([P, 512], F32, tag="ps")
            ps1 = psum_s.tile([P, 512], F32, tag="ps")
            n_acc = NT * len(m1_ops)
            acc = 0
            for kk in range(NT):
                for lo_, ro_ in m1_ops:
                    first = acc == 0
                    last = acc == n_acc - 1
                    lhsT = bcT[:, kk, lo_ + i * P:lo_ + (i + 1) * P]
                    nc.tensor.matmul(ps0[:], lhsT, bcT[:, kk, ro_:ro_ + 512],
                                     start=first, stop=last)
                    nc.tensor.matmul(ps1[:], lhsT,
                                     bcT[:, kk, ro_ + 512:ro_ + 1024],
                                     start=first, stop=last)
                    acc += 1

            m0 = sm.tile([P, 1], F32, tag="sc")
            m1t = sm.tile([P, 1], F32, tag="sc")
            nmx = sm.tile([P, 1], F32, tag="sc")
            nc.vector.tensor_reduce(m0[:], ps0[:], axis=AX.X, op=ALU.max)
            nc.vector.tensor_reduce(m1t[:], ps1[:], axis=AX.X, op=ALU.max)
            nc.vector.tensor_tensor(nmx[:], m0[:], m1t[:], ALU.max)
            nc.vector.tensor_scalar_mul(nmx[:], nmx[:], -1.0)

            E = epool.tile([P, C], F16, tag="E")
            rs0 = sm.tile([P, 1], F32, tag="sc")
            rs1 = sm.tile([P, 1], F32, tag="sc")
            nc.scalar.activation(E[:, 0:512], ps0[:], ACTF.Exp,
                                 bias=nmx[:], scale=1.0, accum_out=rs0[:])
            nc.scalar.activation(E[:, 512:1024], ps1[:], ACTF.Exp,
                                 bias=nmx[:], scale=1.0, accum_out=rs1[:])
            rinv = sm.tile([P, 1], F32, tag="sc")
            nc.vector.tensor_add(rinv[:], rs0[:], rs1[:])
            nc.vector.reciprocal(rinv[:], rinv[:])

            pt = psum_t.tile([P, 8 * P], F16, tag="pt")
            for j in range(8):
                nc.tensor.transpose(
                    pt[:, j * P:(j + 1) * P],
                    E[:, j * P:(j + 1) * P], ident[:])
            nc.vector.tensor_copy(
                ET[:, :, isl],
                pt[:].rearrange("p (t c) -> p t c", t=8))

            po0 = psum_o.tile([P, 512], F32, tag="po")
            po1 = psum_o.tile([P, 512], F32, tag="po")
            for jj in range(NT):
                first, last = jj == 0, jj == NT - 1
                l_e = ET[:, jj, isl]
                nc.tensor.matmul(po0[:], l_e, a16[:, jj, 0:512],
                                 start=first, stop=last)
                nc.tensor.matmul(po1[:], l_e, a16[:, jj, 512:1024],
                                 start=first, stop=last)

            ar = arpool.tile([P, HW], F32, tag="ar")
            nc.scalar.dma_start(ar[:], a_ap[s, isl, :])
            ot = opool.tile([P, HW], F32, tag="ot")
            nc.vector.scalar_tensor_tensor(
                ot[:, 0:512], po0[:], rinv[:], ar[:, 0:512],
                op0=ALU.mult, op1=ALU.add)
            nc.vector.scalar_tensor_tensor(
                ot[:, 512:1024], po1[:], rinv[:], ar[:, 512:1024],
                op0=ALU.mult, op1=ALU.add)
            nc.scalar.dma_start(out_ap[s, isl, :], ot[:])


_BUILT = {}


def build_program(n_samples=S):
    key = (M1_TERMS, n_samples)
    if key in _BUILT:
        return _BUILT[key]
    nc = bacc.Bacc("TRN2", target_bir_lowering=False, debug=False,
                   enable_asserts=False, num_devices=N_CORES)
    a = nc.dram_tensor("a", [S, C, HW], F32, kind="ExternalInput").ap()
    b = nc.dram_tensor("b", [S, C, HW], F32, kind="ExternalInput").ap()
    c = nc.dram_tensor("c", [S, C, HW], F32, kind="ExternalInput").ap()
    out = nc.dram_tensor("out", [S, C, HW], F32, kind="ExternalOutput").ap()
    from contextlib import ExitStack
    with tile.TileContext(nc) as tc, ExitStack() as ctx:
        cam_kernel(ctx, tc, out, a, b, c, n_samples=n_samples)
    nc.compile()
    _BUILT[key] = nc
    return nc


def run_sharded(a, b, c, trace=False, n_samples=S, **kw):
    """a,b,c: [16,1024,1024] fp32 -> (full output, BassKernelResults)."""
    nc = build_program(n_samples)
    in_maps = []
    for core in range(N_CORES):
        sl = slice(core * S, (core + 1) * S)
        in_maps.append({"a": np.ascontiguousarray(a[sl]),
                        "b": np.ascontiguousarray(b[sl]),
                        "c": np.ascontiguousarray(c[sl])})
    res = bass_utils.run_bass_kernel_spmd(
        nc, in_maps, core_ids=list(range(N_CORES)), trace=trace, **kw)
    out = np.concatenate([res.results[core]["out"] for core in range(N_CORES)],
                         axis=0)
    return out, res


def kernel(a, b, c):
    a = np.asarray(a, dtype=np.float32).reshape(B, C, HW)
    b = np.asarray(b, dtype=np.float32).reshape(B, C, HW)
    c = np.asarray(c, dtype=np.float32).reshape(B, C, HW)
    out, _ = run_sharded(a, b, c, trace=False)
    return out.reshape(B, C, H, W)



# revision 3
# speedup vs baseline: 1.0086x; 1.0086x over previous
"""Channel-attention (CAM) Trainium2 kernel.

Problem: out[b] = softmax(b_f[b] @ c_f[b].T, axis=-1) @ a_f[b] + a_f[b]
with a,b,c: [16, 1024, 32, 32] fp32, flattened to [16, 1024, 1024].

Sharding: pure data parallel over batch — 16 samples / 8 cores = 2 per core.

Per-core pipeline (per sample), fp16 compute:
  - b,c loaded via gpsimd cast-DMA (DRAM fp32 -> SBUF fp16, 16-deep
    staging pool so sample s+1 prefetches fully under sample s compute),
    then PE-transposed (fp16) into the merged [HW, C] operand tensor bcT
  - a loaded via cast-DMA directly into its natural-layout fp16 tile
  - software-pipelined i-loop (skew 1): PE order is
      m1(i+1) half0 | E-transpose(i) | m1(i+1) half1 | m2(i)
    so the softmax (DVE reduce + ACT Exp) and the ET psum->SBUF copy of
    step i hide entirely under m1(i+1)'s matmuls
  - m1: scores = bT.T @ cT, fp16, fp32 PSUM accumulation (the two 512-col
    halves are separate psum banks; half0 only needs c row-tiles 0..3)
  - softmax: DVE row-max along free dim, ACT Exp with bias=-max and
    accum_out row-sum; the 1/sum division is deferred to the output
  - m2: out = ET.T @ a16, fp32 PSUM accumulation
  - finalize: DVE scalar_tensor_tensor out = psum * (1/sum) + a16 (the
    residual uses the fp16 a, avoiding a second fp32 load of a), store
    on the sync queue so the scalar queue never blocks the Exps

Note: PE never executes fp32 ops — fp32 transpose-mode matmuls were
observed to hang the PE intermittently when interleaved with 16-bit
FWL-eligible matmul streams.
"""
import sys
import types

import numpy as np


def _install_axon_hooks():
    """Provide antenv.axon_hooks (missing in this image) so trace=True works."""
    if 'antenv.axon_hooks' in sys.modules:
        return
    m = types.ModuleType('antenv.axon_hooks')
    m._hook = None
    m.set_axon_ntff_profile_hook = lambda h: setattr(m, '_hook', h)
    m.get_axon_ntff_profile_hook = lambda: m._hook
    sys.modules['antenv.axon_hooks'] = m
    try:
        import antenv
        antenv.axon_hooks = m
    except ImportError:
        pass
    try:
        from trn_agent_boot.trn_boot import _ntff_profile_via_ctypes
        m.set_axon_ntff_profile_hook(
            _ntff_profile_via_ctypes('/opt/axon/libaxon_pjrt.so'))
    except Exception:
        pass


_install_axon_hooks()

import concourse.bass as bass  # noqa: E402
import concourse.mybir as mybir  # noqa: E402
import concourse.tile as tile  # noqa: E402
from concourse import bacc, bass_utils  # noqa: E402
from concourse.masks import make_identity  # noqa: E402

# artifact upload needs a bucket; keep everything local in the sandbox
bass_utils.upload_artifacts = lambda tmpdir: f"local:{tmpdir}"

N_CORES = 8
B, C, H, W = 16, 1024, 32, 32
HW = H * W
S = B // N_CORES        # samples per core
P = 128
NT = C // P             # 8 row tiles
F32 = mybir.dt.float32
F16 = mybir.dt.float16
ALU = mybir.AluOpType
AX = mybir.AxisListType
ACTF = mybir.ActivationFunctionType


def cam_kernel(ctx, tc, out_ap, a_ap, b_ap, c_ap, n_samples=S):
    nc = tc.nc

    const_pool = ctx.enter_context(tc.tile_pool(name="const", bufs=1))
    natp = ctx.enter_context(tc.tile_pool(name="nat", bufs=16))
    bigp = ctx.enter_context(tc.tile_pool(name="big", bufs=2))
    a16p = ctx.enter_context(tc.tile_pool(name="a16", bufs=2))
    etp = ctx.enter_context(tc.tile_pool(name="et", bufs=2))
    ep = ctx.enter_context(tc.tile_pool(name="E", bufs=2))
    otp = ctx.enter_context(tc.tile_pool(name="ot", bufs=2))
    smp = ctx.enter_context(tc.tile_pool(name="sm", bufs=16))
    pt_pool = ctx.enter_context(tc.tile_pool(name="pt", bufs=2, space="PSUM"))
    ps_pool = ctx.enter_context(tc.tile_pool(name="ps", bufs=4, space="PSUM"))
    po_pool = ctx.enter_context(tc.tile_pool(name="po", bufs=2, space="PSUM"))

    ident = const_pool.tile([P, P], F16)
    make_identity(nc, ident[:])

    for s in range(n_samples):
        bcT = bigp.tile([P, NT, 2 * C], F16, tag="bcT")
        a16 = a16p.tile([P, NT, HW], F16, tag="a16")

        def load_tile(src_ap, r, base, s=s, bcT=bcT):
            rsl = slice(r * P, (r + 1) * P)
            nat = natp.tile([P, HW], F16, tag="nat")
            nc.gpsimd.dma_start(nat[:], src_ap[s, rsl, :])  # cast f32->f16
            pt = pt_pool.tile([P, NT * P], F16, tag="pt")
            for j in range(NT):
                nc.tensor.transpose(
                    pt[:, j * P:(j + 1) * P],
                    nat[:, j * P:(j + 1) * P], ident[:])
            nc.vector.tensor_copy(
                bcT[:, :, base + r * P:base + (r + 1) * P],
                pt[:].rearrange("p (t c) -> p t c", t=NT))

        # order: b row 0 + all of c (m1 i=0 prerequisites), then a (m2
        # prerequisite), then the rest of b (consumed one tile per m1 step)
        load_tile(b_ap, 0, 0)
        for r in range(NT):
            load_tile(c_ap, r, C)
        for r in range(NT):
            nc.gpsimd.dma_start(a16[:, r, :], a_ap[s, r * P:(r + 1) * P, :])
        for r in range(1, NT):
            load_tile(b_ap, r, 0)

        # ---- software-pipelined compute: step i runs m1(i+1) around
        # E-transpose(i)/m2(i) so PE never waits on the softmax chain ----
        ps = {}
        state = {}

        def emit_m1_half(i, half):
            tgt = ps[i][half]
            for kk in range(NT):
                nc.tensor.matmul(
                    tgt[:], bcT[:, kk, i * P:(i + 1) * P],
                    bcT[:, kk, C + half * 512:C + half * 512 + 512],
                    start=(kk == 0), stop=(kk == NT - 1))

        for i in range(-1, NT):
            inx = i + 1
            if inx < NT:
                psa = ps_pool.tile([P, 512], F32, tag="ps")
                psb = ps_pool.tile([P, 512], F32, tag="ps")
                ps[inx] = (psa, psb)
                emit_m1_half(inx, 0)
            if i >= 0:
                # softmax(i) on DVE+ACT (runs while PE does m1(i+1))
                ps0, ps1 = ps.pop(i)
                m0 = smp.tile([P, 1], F32, tag="sc")
                m1t = smp.tile([P, 1], F32, tag="sc")
                nmx = smp.tile([P, 1], F32, tag="sc")
                nc.vector.tensor_reduce(m0[:], ps0[:], axis=AX.X, op=ALU.max)
                nc.vector.tensor_reduce(m1t[:], ps1[:], axis=AX.X, op=ALU.max)
                nc.vector.tensor_tensor(nmx[:], m0[:], m1t[:], ALU.max)
                nc.vector.tensor_scalar_mul(nmx[:], nmx[:], -1.0)

                E = ep.tile([P, C], F16, tag="E")
                rs0 = smp.tile([P, 1], F32, tag="sc")
                rs1 = smp.tile([P, 1], F32, tag="sc")
                nc.scalar.activation(E[:, 0:512], ps0[:], ACTF.Exp,
                                     bias=nmx[:], scale=1.0, accum_out=rs0[:])
                nc.scalar.activation(E[:, 512:1024], ps1[:], ACTF.Exp,
                                     bias=nmx[:], scale=1.0, accum_out=rs1[:])
                rinv = smp.tile([P, 1], F32, tag="sc")
                nc.vector.tensor_add(rinv[:], rs0[:], rs1[:])
                nc.vector.reciprocal(rinv[:], rinv[:])

                # E^T on PE (between the two m1(i+1) halves)
                pt = pt_pool.tile([P, NT * P], F16, tag="pt")
                for j in range(NT):
                    nc.tensor.transpose(
                        pt[:, j * P:(j + 1) * P],
                        E[:, j * P:(j + 1) * P], ident[:])
                ET = etp.tile([P, NT, P], F16, tag="ET")
                nc.vector.tensor_copy(
                    ET[:], pt[:].rearrange("p (t c) -> p t c", t=NT))
                state[i] = (ET, rinv)
            if inx < NT:
                emit_m1_half(inx, 1)
            if i >= 0:
                ET, rinv = state.pop(i)
                po0 = po_pool.tile([P, 512], F32, tag="po")
                po1 = po_pool.tile([P, 512], F32, tag="po")
                for jj in range(NT):
                    first, last = jj == 0, jj == NT - 1
                    l_e = ET[:, jj, :]
                    nc.tensor.matmul(po0[:], l_e, a16[:, jj, 0:512],
                                     start=first, stop=last)
                    nc.tensor.matmul(po1[:], l_e, a16[:, jj, 512:1024],
                                     start=first, stop=last)

                isl = slice(i * P, (i + 1) * P)
                ot = otp.tile([P, HW], F32, tag="ot")
                nc.vector.scalar_tensor_tensor(
                    ot[:, 0:512], po0[:], rinv[:], a16[:, i, 0:512],
                    op0=ALU.mult, op1=ALU.add)
                nc.vector.scalar_tensor_tensor(
                    ot[:, 512:1024], po1[:], rinv[:], a16[:, i, 512:1024],
                    op0=ALU.mult, op1=ALU.add)
                nc.sync.dma_start(out_ap[s, isl, :], ot[:])


_BUILT = {}


def build_program(n_samples=S):
    key = n_samples
    if key in _BUILT:
        return _BUILT[key]
    nc = bacc.Bacc("TRN2", target_bir_lowering=False, debug=False,
                   enable_asserts=False, num_devices=N_CORES)
    a = nc.dram_tensor("a", [S, C, HW], F32, kind="ExternalInput").ap()
    b = nc.dram_tensor("b", [S, C, HW], F32, kind="ExternalInput").ap()
    c = nc.dram_tensor("c", [S, C, HW], F32, kind="ExternalInput").ap()
    out = nc.dram_tensor("out", [S, C, HW], F32, kind="ExternalOutput").ap()
    from contextlib import ExitStack
    with tile.TileContext(nc) as tc, ExitStack() as ctx:
        cam_kernel(ctx, tc, out, a, b, c, n_samples=n_samples)
    nc.compile()
    _BUILT[key] = nc
    return nc


def run_sharded(a, b, c, trace=False, n_samples=S, **kw):
    """a,b,c: [16,1024,1024] fp32 -> (full output, BassKernelResults)."""
    nc = build_program(n_samples)
    in_maps = []
    for core in range(N_CORES):
        sl = slice(core * S, (core + 1) * S)
        in_maps.append({"a": np.ascontiguousarray(a[sl]),
                        "b": np.ascontiguousarray(b[sl]),
                        "c": np.ascontiguousarray(c[sl])})
    res = bass_utils.run_bass_kernel_spmd(
        nc, in_maps, core_ids=list(range(N_CORES)), trace=trace, **kw)
    out = np.concatenate([res.results[core]["out"] for core in range(N_CORES)],
                         axis=0)
    return out, res


def kernel(a, b, c):
    a = np.asarray(a, dtype=np.float32).reshape(B, C, HW)
    b = np.asarray(b, dtype=np.float32).reshape(B, C, HW)
    c = np.asarray(c, dtype=np.float32).reshape(B, C, HW)
    out, _ = run_sharded(a, b, c, trace=False)
    return out.reshape(B, C, H, W)


# revision 4
# speedup vs baseline: 1.1303x; 1.1207x over previous
"""Channel-attention (CAM) Trainium2 kernel.

Problem: out[b] = softmax(b_f[b] @ c_f[b].T, axis=-1) @ a_f[b] + a_f[b]
with a,b,c: [16, 1024, 32, 32] fp32, flattened to [16, 1024, 1024].

Sharding: pure data parallel over batch — 16 samples / 8 cores = 2 per core.

Per-core pipeline (per sample), fp16 compute:
  - b,c loaded via gpsimd cast-DMA (DRAM fp32 -> SBUF fp16, two row-tiles
    per transfer, deep staging pool so sample s+1 prefetches fully under
    sample s compute; s+1's load emission is interleaved into s's compute
    steps so the PE never drains at the sample boundary), then
    PE-transposed (fp16) into the merged [HW, C] operand tensor bcT
  - a loaded via cast-DMA directly into its natural-layout fp16 tile
  - software-pipelined i-loop (skew 1): PE order is
      m1(i+1) kk0-3 | E-transpose(i) | m1(i+1) kk4-7 | m2(i)
    so the softmax (DVE reduce + ACT Exp) and the ET psum->SBUF copy of
    step i hide entirely under m1(i+1)'s matmuls
  - m1: scores = bT.T @ cT, fp16, fp32 PSUM accumulation into a single
    two-bank [128,1024] psum tile (pairs share the stationary operand)
  - softmax: single DVE row-max over 1024, ACT Exp with bias=-max and
    accum_out row-sum; the 1/sum division is deferred to the output
  - m2: out = ET.T @ a16, fp32 PSUM accumulation, single two-bank tile
  - finalize: one DVE scalar_tensor_tensor out = psum * (1/sum) + a16
    (residual uses the fp16 a, avoiding a second fp32 load of a), store
    on the sync queue so the scalar queue never blocks the Exps

Note: PE never executes fp32 ops — fp32 transpose-mode matmuls were
observed to hang the PE intermittently when interleaved with 16-bit
FWL-eligible matmul streams.
"""
import sys
import types

import numpy as np


def _install_axon_hooks():
    """Provide antenv.axon_hooks (missing in this image) so trace=True works."""
    if 'antenv.axon_hooks' in sys.modules:
        return
    m = types.ModuleType('antenv.axon_hooks')
    m._hook = None
    m.set_axon_ntff_profile_hook = lambda h: setattr(m, '_hook', h)
    m.get_axon_ntff_profile_hook = lambda: m._hook
    sys.modules['antenv.axon_hooks'] = m
    try:
        import antenv
        antenv.axon_hooks = m
    except ImportError:
        pass
    try:
        from trn_agent_boot.trn_boot import _ntff_profile_via_ctypes
        m.set_axon_ntff_profile_hook(
            _ntff_profile_via_ctypes('/opt/axon/libaxon_pjrt.so'))
    except Exception:
        pass


_install_axon_hooks()

import concourse.bass as bass  # noqa: E402
import concourse.mybir as mybir  # noqa: E402
import concourse.tile as tile  # noqa: E402
from concourse import bacc, bass_utils  # noqa: E402
from concourse.masks import make_identity  # noqa: E402

# artifact upload needs a bucket; keep everything local in the sandbox
bass_utils.upload_artifacts = lambda tmpdir: f"local:{tmpdir}"

N_CORES = 8
B, C, H, W = 16, 1024, 32, 32
HW = H * W
S = B // N_CORES        # samples per core
P = 128
NT = C // P             # 8 row tiles
F32 = mybir.dt.float32
F16 = mybir.dt.float16
ALU = mybir.AluOpType
AX = mybir.AxisListType
ACTF = mybir.ActivationFunctionType


def cam_kernel(ctx, tc, out_ap, a_ap, b_ap, c_ap, n_samples=S):
    nc = tc.nc

    const_pool = ctx.enter_context(tc.tile_pool(name="const", bufs=1))
    natp = ctx.enter_context(tc.tile_pool(name="nat", bufs=8))
    bigp = ctx.enter_context(tc.tile_pool(name="big", bufs=2))
    a16p = ctx.enter_context(tc.tile_pool(name="a16", bufs=2))
    etp = ctx.enter_context(tc.tile_pool(name="et", bufs=2))
    ep = ctx.enter_context(tc.tile_pool(name="E", bufs=2))
    otp = ctx.enter_context(tc.tile_pool(name="ot", bufs=2))
    smp = ctx.enter_context(tc.tile_pool(name="sm", bufs=16))
    pt_pool = ctx.enter_context(tc.tile_pool(name="pt", bufs=2, space="PSUM"))
    ps_pool = ctx.enter_context(tc.tile_pool(name="ps", bufs=2, space="PSUM"))
    po_pool = ctx.enter_context(tc.tile_pool(name="po", bufs=1, space="PSUM"))

    ident = const_pool.tile([P, P], F16)
    make_identity(nc, ident[:])

    # per-sample persistent tiles
    bcTs = []
    a16s = []
    for s in range(n_samples):
        bcT = bigp.tile([P, NT, 2 * C], F16, tag="bcT", name=f"bcT{s}")
        a16 = a16p.tile([P, NT, HW], F16, tag="a16", name=f"a16{s}")
        bcTs.append(bcT)
        a16s.append(a16)

    def load_pair(s, src_ap, r, base):
        """Cast-load row-tiles r,r+1 of src and transpose into bcT."""
        bcT = bcTs[s]
        nat = natp.tile([P, 2, HW], F16, tag="nat", name=f"nat{s}_{base}_{r}")
        src = src_ap[s, r * P:(r + 2) * P, :].rearrange(
            "(two p) hw -> p two hw", two=2)
        nc.gpsimd.dma_start(nat[:], src)
        for t in range(2):
            pt = pt_pool.tile([P, NT * P], F16, tag="pt", name=f"pt{s}{base}{r}{t}")
            for j in range(NT):
                nc.tensor.transpose(
                    pt[:, j * P:(j + 1) * P],
                    nat[:, t, j * P:(j + 1) * P], ident[:])
            nc.vector.tensor_copy(
                bcT[:, :, base + (r + t) * P:base + (r + t + 1) * P],
                pt[:].rearrange("p (t c) -> p t c", t=NT))

    def load_a(s, r):
        a16 = a16s[s]
        dst = a16[:, r:r + 2, :]
        src = a_ap[s, r * P:(r + 2) * P, :].rearrange(
            "(two p) hw -> p two hw", two=2)
        nc.gpsimd.dma_start(dst, src)

    def emit_loads(s):
        """Yields closures, each emitting one DMA(+transpose) group.

        Order: b r0-1, c r0..7, a r0..7, b r2..7 — m1(i=0) needs b r0 +
        all c; m2(0) needs all a; b r2.. are consumed one per m1 step.
        """
        yield lambda: load_pair(s, b_ap, 0, 0)
        for r in range(0, NT, 2):
            yield (lambda r=r: load_pair(s, c_ap, r, C))
        for r in range(0, NT, 2):
            yield (lambda r=r: load_a(s, r))
        for r in range(2, NT, 2):
            yield (lambda r=r: load_pair(s, b_ap, r, 0))

    def compute_steps(s, next_loads):
        """Emit the 9 pipelined steps for sample s; interleave next
        sample's load emission (2 groups per step) into the steps."""
        bcT = bcTs[s]
        a16 = a16s[s]
        ps = {}
        state = {}

        def emit_m1_kk(i, kk0, kk1):
            tgt = ps[i]
            for kk in range(kk0, kk1):
                lhsT = bcT[:, kk, i * P:(i + 1) * P]
                nc.tensor.matmul(tgt[:, 0:512], lhsT,
                                 bcT[:, kk, C:C + 512],
                                 start=(kk == 0), stop=(kk == NT - 1))
                nc.tensor.matmul(tgt[:, 512:1024], lhsT,
                                 bcT[:, kk, C + 512:C + 1024],
                                 start=(kk == 0), stop=(kk == NT - 1))

        for i in range(-1, NT):
            inx = i + 1
            if inx < NT:
                ps[inx] = ps_pool.tile([P, 1024], F32, tag="ps",
                                       name=f"ps{s}_{inx}")
                emit_m1_kk(inx, 0, 4)
            if i >= 0:
                # softmax(i) on DVE+ACT (runs while PE does m1(i+1))
                pst = ps.pop(i)
                nmx = smp.tile([P, 1], F32, tag="sc", name=f"nmx{s}_{i}")
                nc.vector.tensor_reduce(nmx[:], pst[:], axis=AX.X, op=ALU.max)
                nc.vector.tensor_scalar_mul(nmx[:], nmx[:], -1.0)

                E = ep.tile([P, C], F16, tag="E", name=f"E{s}_{i}")
                rinv = smp.tile([P, 1], F32, tag="sc", name=f"ri{s}_{i}")
                nc.scalar.activation(E[:], pst[:], ACTF.Exp,
                                     bias=nmx[:], scale=1.0, accum_out=rinv[:])
                nc.vector.reciprocal(rinv[:], rinv[:])

                # E^T on PE (between the two m1(i+1) halves)
                pt = pt_pool.tile([P, NT * P], F16, tag="pt", name=f"ptE{s}_{i}")
                for j in range(NT):
                    nc.tensor.transpose(
                        pt[:, j * P:(j + 1) * P],
                        E[:, j * P:(j + 1) * P], ident[:])
                ET = etp.tile([P, NT, P], F16, tag="ET", name=f"ET{s}_{i}")
                nc.vector.tensor_copy(
                    ET[:], pt[:].rearrange("p (t c) -> p t c", t=NT))
                state[i] = (ET, rinv)
            if inx < NT:
                emit_m1_kk(inx, 4, NT)
            if i >= 0:
                ET, rinv = state.pop(i)
                po = po_pool.tile([P, 1024], F32, tag="po", name=f"po{s}_{i}")
                for jj in range(NT):
                    first, last = jj == 0, jj == NT - 1
                    l_e = ET[:, jj, :]
                    nc.tensor.matmul(po[:, 0:512], l_e, a16[:, jj, 0:512],
                                     start=first, stop=last)
                    nc.tensor.matmul(po[:, 512:1024], l_e, a16[:, jj, 512:1024],
                                     start=first, stop=last)

                isl = slice(i * P, (i + 1) * P)
                ot = otp.tile([P, HW], F32, tag="ot", name=f"ot{s}_{i}")
                nc.vector.scalar_tensor_tensor(
                    ot[:], po[:], rinv[:], a16[:, i, :],
                    op0=ALU.mult, op1=ALU.add)
                nc.sync.dma_start(out_ap[s, isl, :], ot[:])
            # interleave next sample's loads (2 groups per step)
            for _ in range(2):
                nl = next(next_loads, None)
                if nl is not None:
                    nl()

    empty = iter(())
    # sample 0: loads upfront (nothing to overlap with)
    for emit in emit_loads(0):
        emit()
    for s in range(n_samples):
        nxt = emit_loads(s + 1) if s + 1 < n_samples else empty
        compute_steps(s, nxt)


_BUILT = {}


def build_program(n_samples=S):
    key = n_samples
    if key in _BUILT:
        return _BUILT[key]
    nc = bacc.Bacc("TRN2", target_bir_lowering=False, debug=False,
                   enable_asserts=False, num_devices=N_CORES)
    a = nc.dram_tensor("a", [S, C, HW], F32, kind="ExternalInput").ap()
    b = nc.dram_tensor("b", [S, C, HW], F32, kind="ExternalInput").ap()
    c = nc.dram_tensor("c", [S, C, HW], F32, kind="ExternalInput").ap()
    out = nc.dram_tensor("out", [S, C, HW], F32, kind="ExternalOutput").ap()
    from contextlib import ExitStack
    with tile.TileContext(nc) as tc, ExitStack() as ctx:
        cam_kernel(ctx, tc, out, a, b, c, n_samples=n_samples)
    nc.compile()
    _BUILT[key] = nc
    return nc


def run_sharded(a, b, c, trace=False, n_samples=S, **kw):
    """a,b,c: [16,1024,1024] fp32 -> (full output, BassKernelResults)."""
    nc = build_program(n_samples)
    in_maps = []
    for core in range(N_CORES):
        sl = slice(core * S, (core + 1) * S)
        in_maps.append({"a": np.ascontiguousarray(a[sl]),
                        "b": np.ascontiguousarray(b[sl]),
                        "c": np.ascontiguousarray(c[sl])})
    res = bass_utils.run_bass_kernel_spmd(
        nc, in_maps, core_ids=list(range(N_CORES)), trace=trace, **kw)
    out = np.concatenate([res.results[core]["out"] for core in range(N_CORES)],
                         axis=0)
    return out, res


def kernel(a, b, c):
    a = np.asarray(a, dtype=np.float32).reshape(B, C, HW)
    b = np.asarray(b, dtype=np.float32).reshape(B, C, HW)
    c = np.asarray(c, dtype=np.float32).reshape(B, C, HW)
    out, _ = run_sharded(a, b, c, trace=False)
    return out.reshape(B, C, H, W)


# revision 6
# speedup vs baseline: 1.1511x; 1.0184x over previous
"""Channel-attention (CAM) Trainium2 kernel.

Problem: out[b] = softmax(b_f[b] @ c_f[b].T, axis=-1) @ a_f[b] + a_f[b]
with a,b,c: [16, 1024, 32, 32] fp32, flattened to [16, 1024, 1024].

Sharding: pure data parallel over batch — 16 samples / 8 cores = 2 per core.

Per-core pipeline (per sample), fp16 compute:
  - b,c loaded via gpsimd cast-DMA (DRAM fp32 -> SBUF fp16, two row-tiles
    per transfer, deep staging pool so sample s+1 prefetches fully under
    sample s compute; s+1's load emission is interleaved into s's compute
    steps so the PE never drains at the sample boundary), then
    PE-transposed (fp16) into the merged [HW, C] operand tensor bcT
  - a loaded via cast-DMA directly into its natural-layout fp16 tile
  - software-pipelined i-loop (skew 1): PE order is
      m1(i+1) kk0-3 | E-transpose(i) | m1(i+1) kk4-7 | m2(i)
    so the softmax (DVE reduce + ACT Exp) and the ET psum->SBUF copy of
    step i hide entirely under m1(i+1)'s matmuls
  - m1: scores = bT.T @ cT, fp16, fp32 PSUM accumulation into a single
    two-bank [128,1024] psum tile (pairs share the stationary operand)
  - softmax: single DVE row-max over 1024, ACT Exp with bias=-max and
    accum_out row-sum; the 1/sum division is deferred to the output
  - m2: out = ET.T @ a16, fp32 PSUM accumulation, single two-bank tile
  - finalize: one DVE scalar_tensor_tensor out = psum * (1/sum) + a16
    (residual uses the fp16 a, avoiding a second fp32 load of a), store
    on the sync queue so the scalar queue never blocks the Exps

Note: PE never executes fp32 ops — fp32 transpose-mode matmuls were
observed to hang the PE intermittently when interleaved with 16-bit
FWL-eligible matmul streams.
"""
import sys
import types

import numpy as np


def _install_axon_hooks():
    """Provide antenv.axon_hooks (missing in this image) so trace=True works."""
    if 'antenv.axon_hooks' in sys.modules:
        return
    m = types.ModuleType('antenv.axon_hooks')
    m._hook = None
    m.set_axon_ntff_profile_hook = lambda h: setattr(m, '_hook', h)
    m.get_axon_ntff_profile_hook = lambda: m._hook
    sys.modules['antenv.axon_hooks'] = m
    try:
        import antenv
        antenv.axon_hooks = m
    except ImportError:
        pass
    try:
        from trn_agent_boot.trn_boot import _ntff_profile_via_ctypes
        m.set_axon_ntff_profile_hook(
            _ntff_profile_via_ctypes('/opt/axon/libaxon_pjrt.so'))
    except Exception:
        pass


_install_axon_hooks()

import concourse.bass as bass  # noqa: E402
import concourse.mybir as mybir  # noqa: E402
import concourse.tile as tile  # noqa: E402
from concourse import bacc, bass_utils  # noqa: E402
from concourse.masks import make_identity  # noqa: E402

# artifact upload needs a bucket; keep everything local in the sandbox
bass_utils.upload_artifacts = lambda tmpdir: f"local:{tmpdir}"

N_CORES = 8
B, C, H, W = 16, 1024, 32, 32
HW = H * W
S = B // N_CORES        # samples per core
P = 128
NT = C // P             # 8 row tiles
F32 = mybir.dt.float32
F16 = mybir.dt.float16
ALU = mybir.AluOpType
AX = mybir.AxisListType
ACTF = mybir.ActivationFunctionType


def cam_kernel(ctx, tc, out_ap, a_ap, b_ap, c_ap, n_samples=S):
    nc = tc.nc

    const_pool = ctx.enter_context(tc.tile_pool(name="const", bufs=1))
    natp = ctx.enter_context(tc.tile_pool(name="nat", bufs=8))
    bigp = ctx.enter_context(tc.tile_pool(name="big", bufs=2))
    a16p = ctx.enter_context(tc.tile_pool(name="a16", bufs=2))
    etp = ctx.enter_context(tc.tile_pool(name="et", bufs=NT + 1))
    ep = ctx.enter_context(tc.tile_pool(name="E", bufs=2))
    otp = ctx.enter_context(tc.tile_pool(name="ot", bufs=2))
    smp = ctx.enter_context(tc.tile_pool(name="sm", bufs=4))
    rip = ctx.enter_context(tc.tile_pool(name="ri", bufs=NT + 1))
    pt_pool = ctx.enter_context(tc.tile_pool(name="pt", bufs=2, space="PSUM"))
    ps_pool = ctx.enter_context(tc.tile_pool(name="ps", bufs=2, space="PSUM"))

    ident = const_pool.tile([P, P], F16)
    make_identity(nc, ident[:])

    # per-sample persistent tiles
    bcTs = []
    a16s = []
    for s in range(n_samples):
        bcT = bigp.tile([P, NT, 2 * C], F16, tag="bcT", name=f"bcT{s}")
        a16 = a16p.tile([P, NT, HW], F16, tag="a16", name=f"a16{s}")
        bcTs.append(bcT)
        a16s.append(a16)

    def load_pair(s, src_ap, r, base):
        """Cast-load row-tiles r,r+1 of src and transpose into bcT."""
        bcT = bcTs[s]
        nat = natp.tile([P, 2, HW], F16, tag="nat", name=f"nat{s}_{base}_{r}")
        src = src_ap[s, r * P:(r + 2) * P, :].rearrange(
            "(two p) hw -> p two hw", two=2)
        nc.gpsimd.dma_start(nat[:], src)
        for t in range(2):
            pt = pt_pool.tile([P, NT * P], F16, tag="pt", name=f"pt{s}{base}{r}{t}")
            for j in range(NT):
                nc.tensor.transpose(
                    pt[:, j * P:(j + 1) * P],
                    nat[:, t, j * P:(j + 1) * P], ident[:])
            nc.vector.tensor_copy(
                bcT[:, :, base + (r + t) * P:base + (r + t + 1) * P],
                pt[:].rearrange("p (t c) -> p t c", t=NT))

    def load_a(s, r):
        a16 = a16s[s]
        dst = a16[:, r:r + 2, :]
        src = a_ap[s, r * P:(r + 2) * P, :].rearrange(
            "(two p) hw -> p two hw", two=2)
        nc.gpsimd.dma_start(dst, src)

    def emit_loads(s):
        """Yields closures, each emitting one DMA(+transpose) group.

        Order: b r0-1, c r0..7, b r2..7, a r0..7 — phase A (m1+softmax)
        needs only b and c; a gates only phase B (m2), which starts
        after all of phase A.
        """
        yield lambda: load_pair(s, b_ap, 0, 0)
        for r in range(0, NT, 2):
            yield (lambda r=r: load_pair(s, c_ap, r, C))
        for r in range(2, NT, 2):
            yield (lambda r=r: load_pair(s, b_ap, r, 0))
        for r in range(0, NT, 2):
            yield (lambda r=r: load_a(s, r))

    def compute_steps(s, next_loads):
        """Phase A: m1 + softmax + ET for all row tiles (skew-1 pipeline);
        phase B: all m2 + finalize back-to-back. The next sample's load
        emission is interleaved into the steps so its DMAs prefetch and
        its transposes fill PE slack."""
        bcT = bcTs[s]
        a16 = a16s[s]
        ps = {}
        state = {}

        def interleave_loads(n):
            for _ in range(n):
                nl = next(next_loads, None)
                if nl is not None:
                    nl()

        def emit_m1_kk(i, kk0, kk1):
            tgt = ps[i]
            for kk in range(kk0, kk1):
                lhsT = bcT[:, kk, i * P:(i + 1) * P]
                nc.tensor.matmul(tgt[:, 0:512], lhsT,
                                 bcT[:, kk, C:C + 512],
                                 start=(kk == 0), stop=(kk == NT - 1))
                nc.tensor.matmul(tgt[:, 512:1024], lhsT,
                                 bcT[:, kk, C + 512:C + 1024],
                                 start=(kk == 0), stop=(kk == NT - 1))

        for i in range(-1, NT):
            inx = i + 1
            if inx < NT:
                ps[inx] = ps_pool.tile([P, 1024], F32, tag="ps",
                                       name=f"ps{s}_{inx}")
                emit_m1_kk(inx, 0, 6)
            if i >= 0:
                # softmax(i) on DVE+ACT (runs while PE does m1(i+1))
                pst = ps.pop(i)
                nmx = smp.tile([P, 1], F32, tag="sc", name=f"nmx{s}_{i}")
                nc.vector.tensor_reduce(nmx[:], pst[:], axis=AX.X, op=ALU.max)
                nc.vector.tensor_scalar_mul(nmx[:], nmx[:], -1.0)

                E = ep.tile([P, C], F16, tag="E", name=f"E{s}_{i}")
                rinv = rip.tile([P, 1], F32, tag="ri", name=f"ri{s}_{i}")
                nc.scalar.activation(E[:], pst[:], ACTF.Exp,
                                     bias=nmx[:], scale=1.0, accum_out=rinv[:])

                # E^T on PE (between the two m1(i+1) chunks)
                pt = pt_pool.tile([P, NT * P], F16, tag="pt", name=f"ptE{s}_{i}")
                for j in range(NT):
                    nc.tensor.transpose(
                        pt[:, j * P:(j + 1) * P],
                        E[:, j * P:(j + 1) * P], ident[:])
                ET = etp.tile([P, NT, P], F16, tag="ET", name=f"ET{s}_{i}")
                nc.vector.tensor_copy(
                    ET[:], pt[:].rearrange("p (t c) -> p t c", t=NT))
                nc.vector.reciprocal(rinv[:], rinv[:])
                state[i] = (ET, rinv)
            if inx < NT:
                emit_m1_kk(inx, 6, NT)
            interleave_loads(1)

        # ---- phase B: m2 burst + finalize ----
        for i in range(NT):
            ET, rinv = state.pop(i)
            po = ps_pool.tile([P, 1024], F32, tag="ps", name=f"po{s}_{i}")
            for jj in range(NT):
                first, last = jj == 0, jj == NT - 1
                l_e = ET[:, jj, :]
                nc.tensor.matmul(po[:, 0:512], l_e, a16[:, jj, 0:512],
                                 start=first, stop=last)
                nc.tensor.matmul(po[:, 512:1024], l_e, a16[:, jj, 512:1024],
                                 start=first, stop=last)

            isl = slice(i * P, (i + 1) * P)
            ot = otp.tile([P, HW], F32, tag="ot", name=f"ot{s}_{i}")
            nc.vector.scalar_tensor_tensor(
                ot[:], po[:], rinv[:], a16[:, i, :],
                op0=ALU.mult, op1=ALU.add)
            nc.sync.dma_start(out_ap[s, isl, :], ot[:])
            interleave_loads(1)

    empty = iter(())
    # sample 0: loads upfront (nothing to overlap with)
    for emit in emit_loads(0):
        emit()
    for s in range(n_samples):
        nxt = emit_loads(s + 1) if s + 1 < n_samples else empty
        compute_steps(s, nxt)


_BUILT = {}


def build_program(n_samples=S):
    key = n_samples
    if key in _BUILT:
        return _BUILT[key]
    nc = bacc.Bacc("TRN2", target_bir_lowering=False, debug=False,
                   enable_asserts=False, num_devices=N_CORES)
    a = nc.dram_tensor("a", [S, C, HW], F32, kind="ExternalInput").ap()
    b = nc.dram_tensor("b", [S, C, HW], F32, kind="ExternalInput").ap()
    c = nc.dram_tensor("c", [S, C, HW], F32, kind="ExternalInput").ap()
    out = nc.dram_tensor("out", [S, C, HW], F32, kind="ExternalOutput").ap()
    from contextlib import ExitStack
    with tile.TileContext(nc) as tc, ExitStack() as ctx:
        cam_kernel(ctx, tc, out, a, b, c, n_samples=n_samples)
    nc.compile()
    _BUILT[key] = nc
    return nc


def run_sharded(a, b, c, trace=False, n_samples=S, **kw):
    """a,b,c: [16,1024,1024] fp32 -> (full output, BassKernelResults)."""
    nc = build_program(n_samples)
    in_maps = []
    for core in range(N_CORES):
        sl = slice(core * S, (core + 1) * S)
        in_maps.append({"a": np.ascontiguousarray(a[sl]),
                        "b": np.ascontiguousarray(b[sl]),
                        "c": np.ascontiguousarray(c[sl])})
    res = bass_utils.run_bass_kernel_spmd(
        nc, in_maps, core_ids=list(range(N_CORES)), trace=trace, **kw)
    out = np.concatenate([res.results[core]["out"] for core in range(N_CORES)],
                         axis=0)
    return out, res


def kernel(a, b, c):
    a = np.asarray(a, dtype=np.float32).reshape(B, C, HW)
    b = np.asarray(b, dtype=np.float32).reshape(B, C, HW)
    c = np.asarray(c, dtype=np.float32).reshape(B, C, HW)
    out, _ = run_sharded(a, b, c, trace=False)
    return out.reshape(B, C, H, W)


# revision 11
# speedup vs baseline: 1.2249x; 1.0641x over previous
"""Channel-attention (CAM) Trainium2 kernel.

Problem: out[b] = softmax(b_f[b] @ c_f[b].T, axis=-1) @ a_f[b] + a_f[b]
with a,b,c: [16, 1024, 32, 32] fp32, flattened to [16, 1024, 1024].

Sharding: pure data parallel over batch — 16 samples / 8 cores = 2 per core.

Per-core pipeline (per sample), fp16 compute:
  - b,c loaded via gpsimd cast-DMA (DRAM fp32 -> SBUF fp16, two row-tiles
    per transfer, deep staging pool so sample s+1 prefetches fully under
    sample s compute; s+1's load emission is interleaved into s's compute
    steps so the PE never drains at the sample boundary), then
    PE-transposed (fp16) into the merged [HW, C] operand tensor bcT
  - a loaded via cast-DMA directly into its natural-layout fp16 tile
  - software-pipelined i-loop (skew 1): PE order is
      m1(i+1) kk0-3 | E-transpose(i) | m1(i+1) kk4-7 | m2(i)
    so the softmax (DVE reduce + ACT Exp) and the ET psum->SBUF copy of
    step i hide entirely under m1(i+1)'s matmuls
  - m1: scores = bT.T @ cT, fp16, fp32 PSUM accumulation into a single
    two-bank [128,1024] psum tile (pairs share the stationary operand)
  - softmax: single DVE row-max over 1024, ACT Exp with bias=-max and
    accum_out row-sum; the 1/sum division is deferred to the output
  - m2: out = ET.T @ a16, fp32 PSUM accumulation, single two-bank tile
  - finalize: one DVE scalar_tensor_tensor out = psum * (1/sum) + a16
    (residual uses the fp16 a, avoiding a second fp32 load of a), store
    on the sync queue so the scalar queue never blocks the Exps

Note: PE never executes fp32 ops — fp32 transpose-mode matmuls were
observed to hang the PE intermittently when interleaved with 16-bit
FWL-eligible matmul streams.
"""
import sys
import types

import numpy as np


def _install_axon_hooks():
    """Provide antenv.axon_hooks (missing in this image) so trace=True works."""
    if 'antenv.axon_hooks' in sys.modules:
        return
    m = types.ModuleType('antenv.axon_hooks')
    m._hook = None
    m.set_axon_ntff_profile_hook = lambda h: setattr(m, '_hook', h)
    m.get_axon_ntff_profile_hook = lambda: m._hook
    sys.modules['antenv.axon_hooks'] = m
    try:
        import antenv
        antenv.axon_hooks = m
    except ImportError:
        pass
    try:
        from trn_agent_boot.trn_boot import _ntff_profile_via_ctypes
        m.set_axon_ntff_profile_hook(
            _ntff_profile_via_ctypes('/opt/axon/libaxon_pjrt.so'))
    except Exception:
        pass


_install_axon_hooks()

import concourse.bass as bass  # noqa: E402
import concourse.mybir as mybir  # noqa: E402
import concourse.tile as tile  # noqa: E402
from concourse import bacc, bass_utils  # noqa: E402
from concourse.masks import make_identity  # noqa: E402

# artifact upload needs a bucket; keep everything local in the sandbox
bass_utils.upload_artifacts = lambda tmpdir: f"local:{tmpdir}"

N_CORES = 8
B, C, H, W = 16, 1024, 32, 32
HW = H * W
S = B // N_CORES        # samples per core
P = 128
NT = C // P             # 8 row tiles
F32 = mybir.dt.float32
F16 = mybir.dt.float16
ALU = mybir.AluOpType
AX = mybir.AxisListType
ACTF = mybir.ActivationFunctionType


def cam_kernel(ctx, tc, out_ap, a_ap, b_ap, c_ap, n_samples=S):
    nc = tc.nc

    const_pool = ctx.enter_context(tc.tile_pool(name="const", bufs=1))
    natp = ctx.enter_context(tc.tile_pool(name="nat", bufs=8))
    bigp = ctx.enter_context(tc.tile_pool(name="big", bufs=2))
    a16p = ctx.enter_context(tc.tile_pool(name="a16", bufs=2))
    etp = ctx.enter_context(tc.tile_pool(name="et", bufs=NT + 1))
    ep = ctx.enter_context(tc.tile_pool(name="E", bufs=2))
    otp = ctx.enter_context(tc.tile_pool(name="ot", bufs=2))
    smp = ctx.enter_context(tc.tile_pool(name="sm", bufs=4))
    rip = ctx.enter_context(tc.tile_pool(name="ri", bufs=NT + 1))
    pt_pool = ctx.enter_context(tc.tile_pool(name="pt", bufs=2, space="PSUM"))
    ps_pool = ctx.enter_context(tc.tile_pool(name="ps", bufs=2, space="PSUM"))

    ident = const_pool.tile([P, P], F16)
    make_identity(nc, ident[:])

    # per-sample persistent tiles
    bcTs = []
    a16s = []
    for s in range(n_samples):
        bcT = bigp.tile([P, NT, 2 * C], F16, tag="bcT", name=f"bcT{s}")
        a16 = a16p.tile([P, NT, HW], F16, tag="a16", name=f"a16{s}")
        bcTs.append(bcT)
        a16s.append(a16)

    def load_pair(s, src_ap, r, base):
        """Cast-load row-tiles r,r+1 of src and transpose into bcT."""
        bcT = bcTs[s]
        nat = natp.tile([P, 2, HW], F16, tag="nat", name=f"nat{s}_{base}_{r}")
        src = src_ap[s, r * P:(r + 2) * P, :].rearrange(
            "(two p) hw -> p two hw", two=2)
        nc.gpsimd.dma_start(nat[:], src)
        for t in range(2):
            pt = pt_pool.tile([P, NT * P], F16, tag="pt", name=f"pt{s}{base}{r}{t}")
            for j in range(NT):
                nc.tensor.transpose(
                    pt[:, j * P:(j + 1) * P],
                    nat[:, t, j * P:(j + 1) * P], ident[:])
            nc.vector.tensor_copy(
                bcT[:, :, base + (r + t) * P:base + (r + t + 1) * P],
                pt[:].rearrange("p (t c) -> p t c", t=NT))

    def load_a(s, r):
        a16 = a16s[s]
        dst = a16[:, r:r + 2, :]
        src = a_ap[s, r * P:(r + 2) * P, :].rearrange(
            "(two p) hw -> p two hw", two=2)
        nc.gpsimd.dma_start(dst, src)

    def bc_loads(s):
        """DMA(+transpose) emitters for b,c: b r0-1 + all c (phase-A
        prerequisites for m1(0..1)), then b r2..7 (one per m1 step)."""
        yield lambda: load_pair(s, b_ap, 0, 0)
        for r in range(0, NT, 2):
            yield (lambda r=r: load_pair(s, c_ap, r, C))
        for r in range(2, NT, 2):
            yield (lambda r=r: load_pair(s, b_ap, r, 0))

    def a_loads(s):
        for r in range(0, NT, 2):
            load_a(s, r)

    def compute_steps(s, next_loads):
        """Phase A: m1 + softmax + ET for all row tiles (skew-1 pipeline);
        phase B: all m2 + finalize back-to-back. The next sample's load
        emission is interleaved into the steps so its DMAs prefetch and
        its transposes fill PE slack."""
        bcT = bcTs[s]
        a16 = a16s[s]
        ps = {}
        state = {}
        own_brest = own_loads.get(s, [])

        def emit_m1_kk(i, kk0, kk1):
            tgt = ps[i]
            for kk in range(kk0, kk1):
                lhsT = bcT[:, kk, i * P:(i + 1) * P]
                nc.tensor.matmul(tgt[:, 0:512], lhsT,
                                 bcT[:, kk, C:C + 512],
                                 start=(kk == 0), stop=(kk == NT - 1))
                nc.tensor.matmul(tgt[:, 512:1024], lhsT,
                                 bcT[:, kk, C + 512:C + 1024],
                                 start=(kk == 0), stop=(kk == NT - 1))

        for i in range(-1, NT):
            inx = i + 1
            if inx < NT:
                ps[inx] = ps_pool.tile([P, 1024], F32, tag="ps",
                                       name=f"ps{s}_{inx}")
                emit_m1_kk(inx, 0, 6)
            if i >= 0:
                # softmax(i) on DVE+ACT (runs while PE does m1(i+1))
                pst = ps.pop(i)
                nmx = smp.tile([P, 1], F32, tag="sc", name=f"nmx{s}_{i}")
                nc.vector.tensor_reduce(nmx[:], pst[:], axis=AX.X, op=ALU.max)
                nc.vector.tensor_scalar_mul(nmx[:], nmx[:], -1.0)

                E = ep.tile([P, C], F16, tag="E", name=f"E{s}_{i}")
                rinv = rip.tile([P, 1], F32, tag="ri", name=f"ri{s}_{i}")
                nc.scalar.activation(E[:], pst[:], ACTF.Exp,
                                     bias=nmx[:], scale=1.0, accum_out=rinv[:])

                # E^T on PE (between the two m1(i+1) chunks)
                pt = pt_pool.tile([P, NT * P], F16, tag="pt", name=f"ptE{s}_{i}")
                for j in range(NT):
                    nc.tensor.transpose(
                        pt[:, j * P:(j + 1) * P],
                        E[:, j * P:(j + 1) * P], ident[:])
                ET = etp.tile([P, NT, P], F16, tag="ET", name=f"ET{s}_{i}")
                nc.vector.tensor_copy(
                    ET[:], pt[:].rearrange("p (t c) -> p t c", t=NT))
                nc.vector.reciprocal(rinv[:], rinv[:])
                state[i] = (ET, rinv)
            if inx < NT:
                emit_m1_kk(inx, 6, NT)
            # own trailing b tiles: transposes paced one per early step,
            # just behind their DMA arrivals (sample 0 only)
            if own_brest:
                own_brest.pop(0)()
                if not own_brest:
                    a_loads(s)   # a: pure DMA issues, gate only phase B

        # ---- phase B: m2 burst + finalize ----
        for i in range(NT):
            ET, rinv = state.pop(i)
            po = ps_pool.tile([P, 1024], F32, tag="ps", name=f"po{s}_{i}")
            for jj in range(NT):
                first, last = jj == 0, jj == NT - 1
                l_e = ET[:, jj, :]
                nc.tensor.matmul(po[:, 0:512], l_e, a16[:, jj, 0:512],
                                 start=first, stop=last)
                nc.tensor.matmul(po[:, 512:1024], l_e, a16[:, jj, 512:1024],
                                 start=first, stop=last)

            isl = slice(i * P, (i + 1) * P)
            ot = otp.tile([P, HW], F32, tag="ot", name=f"ot{s}_{i}")
            nc.vector.scalar_tensor_tensor(
                ot[:], po[:], rinv[:], a16[:, i, :],
                op0=ALU.mult, op1=ALU.add)
            nc.sync.dma_start(out_ap[s, isl, :], ot[:])
            # next sample's b/c groups: one per m2 step (8 groups, 8 steps)
            nl = next(next_loads, None)
            if nl is not None:
                nl()

    # sample 0: b r0-1 + c upfront (nothing to overlap with); its b r2..7
    # are paced into its own early phase-A steps
    own_loads = {}
    l0 = list(bc_loads(0))
    for emit in l0[:5]:
        emit()
    own_loads[0] = l0[5:]
    for s in range(n_samples):
        if s + 1 < n_samples:
            nxt = iter(list(bc_loads(s + 1)))
        else:
            nxt = iter(())
        compute_steps(s, nxt)
        if s + 1 < n_samples:
            for nl in nxt:   # any b/c groups not yet emitted
                nl()
            a_loads(s + 1)


_BUILT = {}


def build_program(n_samples=S):
    key = n_samples
    if key in _BUILT:
        return _BUILT[key]
    nc = bacc.Bacc("TRN2", target_bir_lowering=False, debug=False,
                   enable_asserts=False, num_devices=N_CORES)
    a = nc.dram_tensor("a", [S, C, HW], F32, kind="ExternalInput").ap()
    b = nc.dram_tensor("b", [S, C, HW], F32, kind="ExternalInput").ap()
    c = nc.dram_tensor("c", [S, C, HW], F32, kind="ExternalInput").ap()
    out = nc.dram_tensor("out", [S, C, HW], F32, kind="ExternalOutput").ap()
    from contextlib import ExitStack
    with tile.TileContext(nc) as tc, ExitStack() as ctx:
        cam_kernel(ctx, tc, out, a, b, c, n_samples=n_samples)
    nc.compile()
    _BUILT[key] = nc
    return nc


def run_sharded(a, b, c, trace=False, n_samples=S, **kw):
    """a,b,c: [16,1024,1024] fp32 -> (full output, BassKernelResults)."""
    nc = build_program(n_samples)
    in_maps = []
    for core in range(N_CORES):
        sl = slice(core * S, (core + 1) * S)
        in_maps.append({"a": np.ascontiguousarray(a[sl]),
                        "b": np.ascontiguousarray(b[sl]),
                        "c": np.ascontiguousarray(c[sl])})
    res = bass_utils.run_bass_kernel_spmd(
        nc, in_maps, core_ids=list(range(N_CORES)), trace=trace, **kw)
    out = np.concatenate([res.results[core]["out"] for core in range(N_CORES)],
                         axis=0)
    return out, res


def kernel(a, b, c):
    a = np.asarray(a, dtype=np.float32).reshape(B, C, HW)
    b = np.asarray(b, dtype=np.float32).reshape(B, C, HW)
    c = np.asarray(c, dtype=np.float32).reshape(B, C, HW)
    out, _ = run_sharded(a, b, c, trace=False)
    return out.reshape(B, C, H, W)


# revision 14
# speedup vs baseline: 1.2484x; 1.0192x over previous
"""Channel-attention (CAM) Trainium2 kernel.

Problem: out[b] = softmax(b_f[b] @ c_f[b].T, axis=-1) @ a_f[b] + a_f[b]
with a,b,c: [16, 1024, 32, 32] fp32, flattened to [16, 1024, 1024].

Sharding: pure data parallel over batch — 16 samples / 8 cores = 2 per core.

Per-core pipeline (per sample), fp16 compute:
  - b,c loaded via gpsimd cast-DMA (DRAM fp32 -> SBUF fp16, two row-tiles
    per transfer, deep staging pool so sample s+1 prefetches fully under
    sample s compute; s+1's load emission is interleaved into s's compute
    steps so the PE never drains at the sample boundary), then
    PE-transposed (fp16) into the merged [HW, C] operand tensor bcT
  - a loaded via cast-DMA directly into its natural-layout fp16 tile
  - software-pipelined i-loop (skew 1): PE order is
      m1(i+1) kk0-3 | E-transpose(i) | m1(i+1) kk4-7 | m2(i)
    so the softmax (DVE reduce + ACT Exp) and the ET psum->SBUF copy of
    step i hide entirely under m1(i+1)'s matmuls
  - m1: scores = bT.T @ cT, fp16, fp32 PSUM accumulation into a single
    two-bank [128,1024] psum tile (pairs share the stationary operand)
  - softmax: single DVE row-max over 1024, ACT Exp with bias=-max and
    accum_out row-sum; the 1/sum division is deferred to the output
  - m2: out = ET.T @ a16, fp32 PSUM accumulation, single two-bank tile
  - finalize: one DVE scalar_tensor_tensor out = psum * (1/sum) + a16
    (residual uses the fp16 a, avoiding a second fp32 load of a), store
    on the sync queue so the scalar queue never blocks the Exps

Note: PE never executes fp32 ops — fp32 transpose-mode matmuls were
observed to hang the PE intermittently when interleaved with 16-bit
FWL-eligible matmul streams.
"""
import sys
import types

import numpy as np


def _install_axon_hooks():
    """Provide antenv.axon_hooks (missing in this image) so trace=True works."""
    if 'antenv.axon_hooks' in sys.modules:
        return
    m = types.ModuleType('antenv.axon_hooks')
    m._hook = None
    m.set_axon_ntff_profile_hook = lambda h: setattr(m, '_hook', h)
    m.get_axon_ntff_profile_hook = lambda: m._hook
    sys.modules['antenv.axon_hooks'] = m
    try:
        import antenv
        antenv.axon_hooks = m
    except ImportError:
        pass
    try:
        from trn_agent_boot.trn_boot import _ntff_profile_via_ctypes
        m.set_axon_ntff_profile_hook(
            _ntff_profile_via_ctypes('/opt/axon/libaxon_pjrt.so'))
    except Exception:
        pass


_install_axon_hooks()

import concourse.bass as bass  # noqa: E402
import concourse.mybir as mybir  # noqa: E402
import concourse.tile as tile  # noqa: E402
from concourse import bacc, bass_utils  # noqa: E402
from concourse.masks import make_identity  # noqa: E402

# artifact upload needs a bucket; keep everything local in the sandbox
bass_utils.upload_artifacts = lambda tmpdir: f"local:{tmpdir}"

N_CORES = 8
B, C, H, W = 16, 1024, 32, 32
HW = H * W
S = B // N_CORES        # samples per core
P = 128
NT = C // P             # 8 row tiles
F32 = mybir.dt.float32
F16 = mybir.dt.float16
ALU = mybir.AluOpType
AX = mybir.AxisListType
ACTF = mybir.ActivationFunctionType


def cam_kernel(ctx, tc, out_ap, a_ap, b_ap, c_ap, n_samples=S):
    nc = tc.nc

    const_pool = ctx.enter_context(tc.tile_pool(name="const", bufs=1))
    natp = ctx.enter_context(tc.tile_pool(name="nat", bufs=8))
    bigp = ctx.enter_context(tc.tile_pool(name="big", bufs=2))
    a16p = ctx.enter_context(tc.tile_pool(name="a16", bufs=2))
    etp = ctx.enter_context(tc.tile_pool(name="et", bufs=NT + 1))
    ep = ctx.enter_context(tc.tile_pool(name="E", bufs=2))
    otp = ctx.enter_context(tc.tile_pool(name="ot", bufs=2))
    smp = ctx.enter_context(tc.tile_pool(name="sm", bufs=4))
    rip = ctx.enter_context(tc.tile_pool(name="ri", bufs=NT + 1))
    pt_pool = ctx.enter_context(tc.tile_pool(name="pt", bufs=2, space="PSUM"))
    ps_pool = ctx.enter_context(tc.tile_pool(name="ps", bufs=2, space="PSUM"))

    ident = const_pool.tile([P, P], F16)
    make_identity(nc, ident[:])

    # per-sample persistent tiles
    bcTs = []
    a16s = []
    for s in range(n_samples):
        bcT = bigp.tile([P, NT, 2 * C], F16, tag="bcT", name=f"bcT{s}")
        a16 = a16p.tile([P, NT, HW], F16, tag="a16", name=f"a16{s}")
        bcTs.append(bcT)
        a16s.append(a16)

    def load_pair(s, src_ap, r, base):
        """Cast-load row-tiles r,r+1 of src and transpose into bcT."""
        bcT = bcTs[s]
        nat = natp.tile([P, 2, HW], F16, tag="nat", name=f"nat{s}_{base}_{r}")
        src = src_ap[s, r * P:(r + 2) * P, :].rearrange(
            "(two p) hw -> p two hw", two=2)
        nc.gpsimd.dma_start(nat[:], src)
        for t in range(2):
            pt = pt_pool.tile([P, NT * P], F16, tag="pt", name=f"pt{s}{base}{r}{t}")
            for j in range(NT):
                nc.tensor.transpose(
                    pt[:, j * P:(j + 1) * P],
                    nat[:, t, j * P:(j + 1) * P], ident[:])
            nc.scalar.copy(
                bcT[:, :, base + (r + t) * P:base + (r + t + 1) * P],
                pt[:].rearrange("p (t c) -> p t c", t=NT))

    def load_a(s, r):
        a16 = a16s[s]
        dst = a16[:, r:r + 2, :]
        src = a_ap[s, r * P:(r + 2) * P, :].rearrange(
            "(two p) hw -> p two hw", two=2)
        nc.gpsimd.dma_start(dst, src)

    def bc_loads(s):
        """DMA(+transpose) emitters for b,c: b r0-1 + all c (phase-A
        prerequisites for m1(0..1)), then b r2..7 (one per m1 step)."""
        yield lambda: load_pair(s, b_ap, 0, 0)
        for r in range(0, NT, 2):
            yield (lambda r=r: load_pair(s, c_ap, r, C))
        for r in range(2, NT, 2):
            yield (lambda r=r: load_pair(s, b_ap, r, 0))

    def a_loads(s):
        for r in range(0, NT, 2):
            load_a(s, r)

    def compute_steps(s, next_loads):
        """Phase A: m1 + softmax + ET for all row tiles (skew-1 pipeline);
        phase B: all m2 + finalize back-to-back. The next sample's load
        emission is interleaved into the steps so its DMAs prefetch and
        its transposes fill PE slack."""
        bcT = bcTs[s]
        a16 = a16s[s]
        ps = {}
        state = {}
        own_brest = own_loads.get(s, [])

        def emit_m1_kk(i, kk0, kk1):
            tgt = ps[i]
            for kk in range(kk0, kk1):
                lhsT = bcT[:, kk, i * P:(i + 1) * P]
                nc.tensor.matmul(tgt[:, 0:512], lhsT,
                                 bcT[:, kk, C:C + 512],
                                 start=(kk == 0), stop=(kk == NT - 1))
                nc.tensor.matmul(tgt[:, 512:1024], lhsT,
                                 bcT[:, kk, C + 512:C + 1024],
                                 start=(kk == 0), stop=(kk == NT - 1))

        for i in range(-1, NT):
            inx = i + 1
            if inx < NT:
                ps[inx] = ps_pool.tile([P, 1024], F32, tag="ps",
                                       name=f"ps{s}_{inx}")
                emit_m1_kk(inx, 0, 6)
            if i >= 0:
                # softmax(i) on DVE+ACT (runs while PE does m1(i+1))
                pst = ps.pop(i)
                nmx = smp.tile([P, 1], F32, tag="sc", name=f"nmx{s}_{i}")
                nc.vector.tensor_reduce(nmx[:], pst[:], axis=AX.X, op=ALU.max)
                nc.vector.tensor_scalar_mul(nmx[:], nmx[:], -1.0)

                E = ep.tile([P, C], F16, tag="E", name=f"E{s}_{i}")
                rinv = rip.tile([P, 1], F32, tag="ri", name=f"ri{s}_{i}")
                nc.scalar.activation(E[:], pst[:], ACTF.Exp,
                                     bias=nmx[:], scale=1.0, accum_out=rinv[:])

                # E^T on PE (between the two m1(i+1) chunks)
                pt = pt_pool.tile([P, NT * P], F16, tag="pt", name=f"ptE{s}_{i}")
                for j in range(NT):
                    nc.tensor.transpose(
                        pt[:, j * P:(j + 1) * P],
                        E[:, j * P:(j + 1) * P], ident[:])
                ET = etp.tile([P, NT, P], F16, tag="ET", name=f"ET{s}_{i}")
                nc.scalar.copy(
                    ET[:], pt[:].rearrange("p (t c) -> p t c", t=NT))
                nc.vector.reciprocal(rinv[:], rinv[:])
                state[i] = (ET, rinv)
            if inx < NT:
                emit_m1_kk(inx, 6, NT)
            # own trailing b tiles: transposes paced one per early step,
            # just behind their DMA arrivals (sample 0 only)
            if own_brest:
                own_brest.pop(0)()
                if not own_brest:
                    a_loads(s)   # a: pure DMA issues, gate only phase B

        # ---- phase B: m2 burst + finalize ----
        for i in range(NT):
            ET, rinv = state.pop(i)
            po = ps_pool.tile([P, 1024], F32, tag="ps", name=f"po{s}_{i}")
            for jj in range(NT):
                first, last = jj == 0, jj == NT - 1
                l_e = ET[:, jj, :]
                nc.tensor.matmul(po[:, 0:512], l_e, a16[:, jj, 0:512],
                                 start=first, stop=last)
                nc.tensor.matmul(po[:, 512:1024], l_e, a16[:, jj, 512:1024],
                                 start=first, stop=last)

            isl = slice(i * P, (i + 1) * P)
            ot = otp.tile([P, HW], F32, tag="ot", name=f"ot{s}_{i}")
            if s == n_samples - 1 and i == NT - 1:
                # last tile: split finalize+store in halves to shorten the tail
                for h in range(2):
                    hsl = slice(h * 512, (h + 1) * 512)
                    nc.vector.scalar_tensor_tensor(
                        ot[:, hsl], po[:, hsl], rinv[:], a16[:, i, hsl],
                        op0=ALU.mult, op1=ALU.add)
                    nc.sync.dma_start(out_ap[s, isl, hsl], ot[:, hsl])
            else:
                nc.vector.scalar_tensor_tensor(
                    ot[:], po[:], rinv[:], a16[:, i, :],
                    op0=ALU.mult, op1=ALU.add)
                nc.sync.dma_start(out_ap[s, isl, :], ot[:])
            # next sample's b/c groups: one per m2 step (8 groups, 8 steps)
            nl = next(next_loads, None)
            if nl is not None:
                nl()

    # sample 0: b r0-1 + c upfront (nothing to overlap with); its b r2..7
    # are paced into its own early phase-A steps
    own_loads = {}
    l0 = list(bc_loads(0))
    for emit in l0[:5]:
        emit()
    own_loads[0] = l0[5:]
    for s in range(n_samples):
        if s + 1 < n_samples:
            nxt = iter(list(bc_loads(s + 1)))
        else:
            nxt = iter(())
        compute_steps(s, nxt)
        if s + 1 < n_samples:
            for nl in nxt:   # any b/c groups not yet emitted
                nl()
            a_loads(s + 1)


_BUILT = {}


def build_program(n_samples=S):
    key = n_samples
    if key in _BUILT:
        return _BUILT[key]
    nc = bacc.Bacc("TRN2", target_bir_lowering=False, debug=False,
                   enable_asserts=False, num_devices=N_CORES)
    a = nc.dram_tensor("a", [S, C, HW], F32, kind="ExternalInput").ap()
    b = nc.dram_tensor("b", [S, C, HW], F32, kind="ExternalInput").ap()
    c = nc.dram_tensor("c", [S, C, HW], F32, kind="ExternalInput").ap()
    out = nc.dram_tensor("out", [S, C, HW], F32, kind="ExternalOutput").ap()
    from contextlib import ExitStack
    with tile.TileContext(nc) as tc, ExitStack() as ctx:
        cam_kernel(ctx, tc, out, a, b, c, n_samples=n_samples)
    nc.compile()
    _BUILT[key] = nc
    return nc


def run_sharded(a, b, c, trace=False, n_samples=S, **kw):
    """a,b,c: [16,1024,1024] fp32 -> (full output, BassKernelResults)."""
    nc = build_program(n_samples)
    in_maps = []
    for core in range(N_CORES):
        sl = slice(core * S, (core + 1) * S)
        in_maps.append({"a": np.ascontiguousarray(a[sl]),
                        "b": np.ascontiguousarray(b[sl]),
                        "c": np.ascontiguousarray(c[sl])})
    res = bass_utils.run_bass_kernel_spmd(
        nc, in_maps, core_ids=list(range(N_CORES)), trace=trace, **kw)
    out = np.concatenate([res.results[core]["out"] for core in range(N_CORES)],
                         axis=0)
    return out, res


def kernel(a, b, c):
    a = np.asarray(a, dtype=np.float32).reshape(B, C, HW)
    b = np.asarray(b, dtype=np.float32).reshape(B, C, HW)
    c = np.asarray(c, dtype=np.float32).reshape(B, C, HW)
    out, _ = run_sharded(a, b, c, trace=False)
    return out.reshape(B, C, H, W)
